# revision 2
# baseline (speedup 1.0000x reference)
"""Self-contained Trainium2 Bass kernel: GRU(relu, reset_after) + BN + Dense.

kernel(**inputs) takes FULL unsharded fp32 inputs, shards batch over 8
NeuronCores, runs the Bass kernel via run_bass_kernel_spmd, returns the
FULL [2048, 1] fp32 output.

Layout (per core):
  B=256 batch (2 chunks of Bc=128), T timesteps, F=32 in-features, H=256 hidden.
  Transposed: hidden on partitions, batch on free dim.
  h[c][p, ct*128+n] = h_state[batch c*128+n, hidden ct*128+p]

DRAM inputs (host-prepped):
  xT  [T/2*128, 256] f16  tile j: rows 0:32 = x[2j] feats, row 32 = 1.0,
                          rows 64:96 = x[2j+1] feats, row 96 = 1.0, rest 0.
  wi  [128, 1024] f16     rows 0:32 cols 0:768 = kernel (gate block m at
                          cols m*128); row 32 cols 0:768 = [b_z|b_r|b_xh];
                          row 32 cols 768:1024 = [b_rh0|b_rh1]; rows 64:97
                          replicate rows 0:33 (for odd-timestep row group).
  wr8 [128, 1536] f8e4    rec weights: [:, m*256+k*128+j] = rec[k*128+p, m*128+j]
                          (DoubleRow k-tile layout, K=256 in one matmul)
  sv  [128, 2] f16        BN+dense folded scale; cv [1,1] f32 folded bias.
Output:
  y   [1, 256] f32        per-core output slice.

All gate biases are baked into the PSUM accumulation by the x-projection
matmuls (K=33: 32 features + a ones-row whose stationary row carries the
bias), and b_rh via two K=1 matmuls — no bias work on vector engines.

Gate math per step (z/r/xh/rh pre-acts accumulated in PSUM):
  r = sigmoid(psum_r); z = sigmoid(psum_z); w = sigmoid(-psum_z) = 1-z
  p = psum_rh * r ; s = psum_xh + p
  u = z*h ; v = relu(s)*w ; h_new = v + u ; h8 = fp8(h_new)
"""
from contextlib import ExitStack

import numpy as np

import concourse.bass as bass
import concourse.tile as tile
from concourse import bacc, mybir

F16 = mybir.dt.float16
F32 = mybir.dt.float32
F8 = mybir.dt.float8e4
AF = mybir.ActivationFunctionType
OP = mybir.AluOpType
DR = mybir.MatmulPerfMode.DoubleRow


def build_gru_nc(T=256, debug=False):
    nc = bacc.Bacc("TRN2", num_devices=8, debug=debug)
    xT_d = nc.dram_tensor("xT", [T // 2 * 128, 256], F16, kind="ExternalInput")
    wi_d = nc.dram_tensor("wi", [128, 1024], F16, kind="ExternalInput")
    wr8_d = nc.dram_tensor("wr8", [128, 1536], F8, kind="ExternalInput")
    sv_d = nc.dram_tensor("sv", [128, 2], F16, kind="ExternalInput")
    cv_d = nc.dram_tensor("cv", [1, 1], F32, kind="ExternalInput")
    y_d = nc.dram_tensor("y", [1, 256], F32, kind="ExternalOutput")

    with tile.TileContext(nc) as tc, ExitStack() as ctx:
        const = ctx.enter_context(tc.tile_pool(name="const", bufs=1))
        hpool = [
            ctx.enter_context(tc.tile_pool(name=f"h{c}", bufs=2)) for c in (0, 1)
        ]
        h8pool = [
            ctx.enter_context(tc.tile_pool(name=f"h8{c}", bufs=2)) for c in (0, 1)
        ]
        gpool = [
            ctx.enter_context(tc.tile_pool(name=f"g{c}", bufs=2)) for c in (0, 1)
        ]
        zrpool = [
            ctx.enter_context(
                tc.tile_pool(name=f"zr{c}", bufs=2, space=bass.MemorySpace.PSUM)
            )
            for c in (0, 1)
        ]
        xrpool = [
            ctx.enter_context(
                tc.tile_pool(name=f"xr{c}", bufs=2, space=bass.MemorySpace.PSUM)
            )
            for c in (0, 1)
        ]

        ntile = T // 2  # [128, 256] x-tiles, one per 2 timesteps
        xsb = const.tile([128, ntile * 256], F16)
        wi = const.tile([128, 1024], F16)
        wr8 = const.tile([128, 1536], F8)
        sv = const.tile([128, 2], F16)
        cv = const.tile([1, 1], F32)

        nc.sync.dma_start(wi[:], wi_d.ap())
        nc.sync.dma_start(wr8[:], wr8_d.ap())
        nc.sync.dma_start(sv[:], sv_d.ap())
        nc.sync.dma_start(cv[:], cv_d.ap())

        nchunk = 4
        per = ntile // nchunk
        for jc in range(nchunk):
            src = xT_d.ap()[jc * per * 128 : (jc + 1) * per * 128, :]
            src = src.rearrange("(j p) b -> p j b", p=128)
            dst = xsb[:, jc * per * 256 : (jc + 1) * per * 256]
            dst = dst.rearrange("p (j b) -> p j b", b=256)
            nc.sync.dma_start(dst, src)

        h, h8 = [], []
        for c in (0, 1):
            h0 = hpool[c].tile([128, 256], F16)
            nc.vector.memset(h0[:], 0.0)
            h.append(h0)
            h80 = h8pool[c].tile([128, 256], F8)
            nc.gpsimd.memset(h80[:], 0.0)
            h8.append(h80)

        def x_phase(t, c):
            """x-projection + all-bias matmuls for step t, chunk c."""
            zr = zrpool[c].tile([128, 512], F32)
            xr = xrpool[c].tile([128, 512], F32)
            base = 64 * (t % 2)
            col0 = (t // 2) * 256 + c * 128
            xrhs = xsb[base : base + 33, col0 : col0 + 128]
            for m in range(6):
                lhsT = wi[base : base + 33, m * 128 : (m + 1) * 128]
                if m < 4:
                    out = zr[:, m * 128 : (m + 1) * 128]
                else:
                    out = xr[:, (m - 4) * 128 : (m - 3) * 128]
                nc.tensor.matmul(
                    out, lhsT, xrhs, start=(m in (0, 4)), stop=False,
                    tile_position=(base, 0),
                )
            # b_rh into the rh region via K=1 mm against the ones-row
            ones = xsb[base + 32 : base + 33, col0 : col0 + 128]
            for ct in (0, 1):
                lhsT = wi[base + 32 : base + 33, 768 + ct * 128 : 768 + (ct + 1) * 128]
                nc.tensor.matmul(
                    xr[:, 256 + ct * 128 : 256 + (ct + 1) * 128], lhsT, ones,
                    start=False, stop=False, tile_position=(base + 32, 0),
                )
            return zr, xr

        cur = [x_phase(0, 0), x_phase(0, 1)]

        for t in range(T):
            for c in (0, 1):
                zr, xr = cur[c]
                hc, h8c = h[c], h8[c]
                rhs8 = h8c[:].rearrange("p (k n) -> p k n", k=2)
                # rec matmuls, fp8 DoubleRow (K=256 in one pass):
                # r blocks (2,3) first so sigmoid(r) starts early, z (0,1),
                # then rh (4,5).
                for m in (2, 3, 0, 1, 4, 5):
                    if m < 4:
                        out = zr[:, m * 128 : (m + 1) * 128]
                    else:
                        out = xr[:, 256 + (m - 4) * 128 : 256 + (m - 3) * 128]
                    lhsT = wr8[:, m * 256 : (m + 1) * 256].rearrange(
                        "p (k j) -> p k j", k=2
                    )
                    nc.tensor.matmul(
                        out, lhsT, rhs8,
                        start=False, stop=(m in (1, 5)), perf_mode=DR,
                    )

                r_sb = gpool[c].tile([128, 256], F16, tag="r")
                z_sb = gpool[c].tile([128, 256], F16, tag="z")
                w_sb = gpool[c].tile([128, 256], F16, tag="w")
                nc.scalar.activation(r_sb[:], zr[:, 256:512], AF.Sigmoid)
                nc.scalar.activation(z_sb[:], zr[:, 0:256], AF.Sigmoid)
                nc.scalar.activation(w_sb[:], zr[:, 0:256], AF.Sigmoid, scale=-1.0)

                p = gpool[c].tile([128, 256], F16, tag="p")
                nc.vector.tensor_tensor(p[:], xr[:, 256:512], r_sb[:], OP.mult)
                s = gpool[c].tile([128, 256], F16, tag="s")
                nc.vector.tensor_tensor(s[:], xr[:, 0:256], p[:], OP.add)
                u = gpool[c].tile([128, 256], F16, tag="u")
                nc.gpsimd.tensor_tensor(u[:], z_sb[:], hc[:], OP.mult)
                v = gpool[c].tile([128, 256], F16, tag="v")
                nc.vector.scalar_tensor_tensor(
                    v[:], s[:], 0.0, w_sb[:], op0=OP.max, op1=OP.mult
                )
                hn = hpool[c].tile([128, 256], F16)
                nc.gpsimd.tensor_tensor(hn[:], v[:], u[:], OP.add)
                h8n = h8pool[c].tile([128, 256], F8)
                nc.gpsimd.tensor_copy(h8n[:], hn[:])
                h[c] = hn
                h8[c] = h8n

                if t + 1 < T:
                    cur[c] = x_phase(t + 1, c)

        # ---- BN + dense epilogue: y = s . h + c ----
        fin = zrpool[0].tile([128, 512], F32, tag="zr")
        first = True
        for c in (0, 1):
            for ct in (0, 1):
                nc.tensor.matmul(
                    fin[0:1, c * 128 : (c + 1) * 128],
                    sv[:, ct : ct + 1],
                    h[c][:, ct * 128 : (ct + 1) * 128],
                    start=first,
                    stop=(c == 1 and ct == 1),
                )
                first = False
        ysb = const.tile([1, 256], F32)
        nc.vector.tensor_scalar_add(ysb[:], fin[0:1, 0:256], cv[0:1, 0:1])
        nc.sync.dma_start(y_d.ap(), ysb[:])

    nc.compile()
    return nc


BN_EPS = 1e-3


def prep_core_inputs(x_core, kernel, rec_kernel, bias, gamma, beta,
                     moving_mean, moving_var, dense_w, dense_b):
    """Host-side prep of one core's input dict. x_core: [B=256, T, 32] f32."""
    import ml_dtypes

    B, T, F = x_core.shape
    H = 256
    # xT tiles: 2 timesteps per 128 rows, feats at rows 0:32 / 64:96,
    # ones-row at 32 / 96.
    xt = np.zeros((T // 2, 128, B), np.float16)
    xf = x_core.astype(np.float16).transpose(1, 2, 0)  # [T, 32, B]
    xt[:, 0:32, :] = xf[0::2]
    xt[:, 64:96, :] = xf[1::2]
    xt[:, 32, :] = 1.0
    xt[:, 96, :] = 1.0
    xT = np.ascontiguousarray(xt.reshape(T // 2 * 128, B))

    b_z = bias[0, 0:256] + bias[1, 0:256]
    b_r = bias[0, 256:512] + bias[1, 256:512]
    b_xh = bias[0, 512:768]
    b_rh = bias[1, 512:768]
    wi = np.zeros((128, 1024), np.float16)
    wi[0:32, 0:768] = kernel.astype(np.float16)
    wi[32, 0:768] = np.concatenate([b_z, b_r, b_xh]).astype(np.float16)
    wi[32, 768:1024] = b_rh.astype(np.float16)
    wi[64:97, :] = wi[0:33, :]

    rec8 = np.clip(rec_kernel, -240, 240).astype(ml_dtypes.float8_e4m3)
    wr8 = np.zeros((128, 1536), ml_dtypes.float8_e4m3)
    for m in range(6):
        for k in (0, 1):
            wr8[:, m * 256 + k * 128 : m * 256 + (k + 1) * 128] = rec8[
                k * 128 : (k + 1) * 128, m * 128 : (m + 1) * 128
            ]

    rs = 1.0 / np.sqrt(moving_var + BN_EPS)
    s = (gamma * rs * dense_w[:, 0]).astype(np.float16)
    sv = np.stack([s[:128], s[128:]], axis=1)
    cc = dense_b[0] + np.sum((beta - moving_mean * gamma * rs) * dense_w[:, 0])
    cv = np.array([[cc]], np.float32)
    return {
        "xT": xT,
        "wi": np.ascontiguousarray(wi),
        "wr8": np.ascontiguousarray(wr8),
        "sv": np.ascontiguousarray(sv),
        "cv": cv,
    }


_NC_CACHE = {}


def _get_nc():
    if "nc" not in _NC_CACHE:
        _NC_CACHE["nc"] = build_gru_nc(T=256)
    return _NC_CACHE["nc"]


def kernel(x, kernel, rec_kernel, bias, gamma, beta, moving_mean, moving_var,
           dense_w, dense_b):
    from concourse.bass_utils import run_bass_kernel_spmd

    x = np.asarray(x, dtype=np.float32)
    args = [np.asarray(a, dtype=np.float32) for a in
            (kernel, rec_kernel, bias, gamma, beta, moving_mean, moving_var,
             dense_w, dense_b)]
    nc = _get_nc()
    n_cores = 8
    nb = x.shape[0] // n_cores
    in_maps = [prep_core_inputs(x[i * nb : (i + 1) * nb], *args)
               for i in range(n_cores)]
    res = run_bass_kernel_spmd(nc, in_maps, core_ids=list(range(n_cores)))
    return np.concatenate(
        [res.results[i]["y"].reshape(nb, 1) for i in range(n_cores)], axis=0
    ).astype(np.float32)


# revision 3
# speedup vs baseline: 1.1603x; 1.1603x over previous
"""Self-contained Trainium2 Bass kernel: GRU(relu, reset_after) + BN + Dense.

kernel(**inputs) takes FULL unsharded fp32 inputs, shards batch over 8
NeuronCores, runs the Bass kernel via run_bass_kernel_spmd, returns the
FULL [2048, 1] fp32 output.

Layout (per core):
  B=256 batch (2 chunks of Bc=128), T timesteps, F=32 in-features, H=256 hidden.
  Transposed: hidden on partitions, batch on free dim.
  h[c][p, ct*128+n] = h_state[batch c*128+n, hidden ct*128+p]

DRAM inputs (host-prepped):
  xT  [T/2*128, 256] f16  tile j: rows 0:32 = x[2j] feats, row 32 = 1.0,
                          rows 64:96 = x[2j+1] feats, row 96 = 1.0, rest 0.
  wi  [128, 1024] f16     rows 0:32 cols 0:768 = kernel (gate block m at
                          cols m*128); row 32 cols 0:768 = [b_z|b_r|b_xh];
                          row 32 cols 768:1024 = [b_rh0|b_rh1]; rows 64:97
                          replicate rows 0:33 (for odd-timestep row group).
  wr8 [128, 1536] f8e4    rec weights: [:, m*256+k*128+j] = rec[k*128+p, m*128+j]
                          (DoubleRow k-tile layout, K=256 in one matmul)
  sv  [128, 2] f16        BN+dense folded scale; cv [1,1] f32 folded bias.
Output:
  y   [1, 256] f32        per-core output slice.

All gate biases are baked into the PSUM accumulation by the x-projection
matmuls (K=33: 32 features + a ones-row whose stationary row carries the
bias), and b_rh via two K=1 matmuls — no bias work on vector engines.

Gate math per step (z/r/xh/rh pre-acts accumulated in PSUM):
  r = sigmoid(psum_r); z = sigmoid(psum_z); w = sigmoid(-psum_z) = 1-z
  p = psum_rh * r ; s = psum_xh + p
  u = z*h ; v = relu(s)*w ; h_new = v + u ; h8 = fp8(h_new)
"""
from contextlib import ExitStack

import numpy as np

import concourse.bass as bass
import concourse.tile as tile
from concourse import bacc, mybir

F16 = mybir.dt.float16
F32 = mybir.dt.float32
F8 = mybir.dt.float8e4
AF = mybir.ActivationFunctionType
OP = mybir.AluOpType
DR = mybir.MatmulPerfMode.DoubleRow


def build_gru_nc(T=256, debug=False):
    nc = bacc.Bacc("TRN2", num_devices=8, debug=debug)
    xT_d = nc.dram_tensor("xT", [T // 2 * 128, 256], F16, kind="ExternalInput")
    wi_d = nc.dram_tensor("wi", [128, 1024], F16, kind="ExternalInput")
    wr8_d = nc.dram_tensor("wr8", [128, 1536], F8, kind="ExternalInput")
    sv_d = nc.dram_tensor("sv", [128, 2], F16, kind="ExternalInput")
    cv_d = nc.dram_tensor("cv", [1, 1], F32, kind="ExternalInput")
    y_d = nc.dram_tensor("y", [1, 256], F32, kind="ExternalOutput")

    with tile.TileContext(nc) as tc, ExitStack() as ctx:
        const = ctx.enter_context(tc.tile_pool(name="const", bufs=1))
        hpool = [
            ctx.enter_context(tc.tile_pool(name=f"h{c}", bufs=2)) for c in (0, 1)
        ]
        h8pool = [
            ctx.enter_context(tc.tile_pool(name=f"h8{c}", bufs=2)) for c in (0, 1)
        ]
        gpool = [
            ctx.enter_context(tc.tile_pool(name=f"g{c}", bufs=2)) for c in (0, 1)
        ]
        zrpool = [
            ctx.enter_context(
                tc.tile_pool(name=f"zr{c}", bufs=2, space=bass.MemorySpace.PSUM)
            )
            for c in (0, 1)
        ]
        xrpool = [
            ctx.enter_context(
                tc.tile_pool(name=f"xr{c}", bufs=2, space=bass.MemorySpace.PSUM)
            )
            for c in (0, 1)
        ]

        ntile = T // 2  # [128, 256] x-tiles, one per 2 timesteps
        xsb = const.tile([128, ntile * 256], F16)
        wi = const.tile([128, 1024], F16)
        wr8 = const.tile([128, 1536], F8)
        sv = const.tile([128, 2], F16)
        cv = const.tile([1, 1], F32)

        nc.sync.dma_start(wi[:], wi_d.ap())
        nc.sync.dma_start(wr8[:], wr8_d.ap())
        nc.sync.dma_start(sv[:], sv_d.ap())
        nc.sync.dma_start(cv[:], cv_d.ap())

        nchunk = 4
        per = ntile // nchunk
        for jc in range(nchunk):
            src = xT_d.ap()[jc * per * 128 : (jc + 1) * per * 128, :]
            src = src.rearrange("(j p) b -> p j b", p=128)
            dst = xsb[:, jc * per * 256 : (jc + 1) * per * 256]
            dst = dst.rearrange("p (j b) -> p j b", b=256)
            nc.sync.dma_start(dst, src)

        h, h8 = [], []
        for c in (0, 1):
            h0 = hpool[c].tile([128, 256], F16)
            nc.vector.memset(h0[:], 0.0)
            h.append(h0)
            h80 = h8pool[c].tile([128, 256], F8)
            nc.gpsimd.memset(h80[:], 0.0)
            h8.append(h80)

        def x_phase(t, c):
            """x-projection + all-bias matmuls for step t, chunk c."""
            zr = zrpool[c].tile([128, 512], F32)
            xr = xrpool[c].tile([128, 512], F32)
            base = 64 * (t % 2)
            col0 = (t // 2) * 256 + c * 128
            xrhs = xsb[base : base + 33, col0 : col0 + 128]
            for m in range(6):
                lhsT = wi[base : base + 33, m * 128 : (m + 1) * 128]
                if m < 4:
                    out = zr[:, m * 128 : (m + 1) * 128]
                else:
                    out = xr[:, (m - 4) * 128 : (m - 3) * 128]
                nc.tensor.matmul(
                    out, lhsT, xrhs, start=(m in (0, 4)), stop=False,
                    tile_position=(base, 0),
                )
            # b_rh into the rh region via K=1 mm against the ones-row
            ones = xsb[base + 32 : base + 33, col0 : col0 + 128]
            for ct in (0, 1):
                lhsT = wi[base + 32 : base + 33, 768 + ct * 128 : 768 + (ct + 1) * 128]
                nc.tensor.matmul(
                    xr[:, 256 + ct * 128 : 256 + (ct + 1) * 128], lhsT, ones,
                    start=False, stop=False, tile_position=(base + 32, 0),
                )
            return zr, xr

        cur = [x_phase(0, 0), x_phase(0, 1)]
        pending_cast = None  # (hn_tile, chunk) awaiting fp8 cast on scalar

        for t in range(T):
            for c in (0, 1):
                zr, xr = cur[c]
                hc, h8c = h[c], h8[c]
                rhs8 = h8c[:].rearrange("p (k n) -> p k n", k=2)
                # rec matmuls, fp8 DoubleRow (K=256 in one pass): z/r blocks
                # first so the fused sigmoid starts early, then rh (4,5).
                for m in (0, 1, 2, 3, 4, 5):
                    if m < 4:
                        out = zr[:, m * 128 : (m + 1) * 128]
                    else:
                        out = xr[:, 256 + (m - 4) * 128 : 256 + (m - 3) * 128]
                    lhsT = wr8[:, m * 256 : (m + 1) * 256].rearrange(
                        "p (k j) -> p k j", k=2
                    )
                    nc.tensor.matmul(
                        out, lhsT, rhs8,
                        start=False, stop=(m in (3, 5)), perf_mode=DR,
                    )

                # fused sigmoid over the whole z|r bank: z = [:,0:256],
                # r = [:,256:512]
                zr_sb = gpool[c].tile([128, 512], F16, tag="zr")
                nc.scalar.activation(zr_sb[:], zr[:, 0:512], AF.Sigmoid)
                z_sb, r_sb = zr_sb[:, 0:256], zr_sb[:, 256:512]
                # deferred fp8 cast of the OTHER chunk's h (after this
                # sigmoid in the in-order scalar queue, so it never blocks it)
                if pending_cast is not None:
                    hn_prev, cprev = pending_cast
                    h8n = h8pool[cprev].tile([128, 256], F8)
                    nc.scalar.copy(h8n[:], hn_prev[:])
                    h8[cprev] = h8n
                    pending_cast = None

                w_sb = gpool[c].tile([128, 256], F16, tag="w")
                nc.gpsimd.tensor_scalar(w_sb[:], z_sb, -1.0, 1.0, OP.mult, OP.add)
                u = gpool[c].tile([128, 256], F16, tag="u")
                nc.gpsimd.tensor_tensor(u[:], z_sb, hc[:], OP.mult)

                p = gpool[c].tile([128, 256], F16, tag="p")
                nc.vector.tensor_tensor(p[:], xr[:, 256:512], r_sb, OP.mult)
                s = gpool[c].tile([128, 256], F16, tag="s")
                nc.vector.tensor_tensor(s[:], xr[:, 0:256], p[:], OP.add)
                v = gpool[c].tile([128, 256], F16, tag="v")
                nc.vector.scalar_tensor_tensor(
                    v[:], s[:], 0.0, w_sb[:], op0=OP.max, op1=OP.mult
                )
                hn = hpool[c].tile([128, 256], F16)
                nc.vector.tensor_tensor(hn[:], v[:], u[:], OP.add)
                h[c] = hn
                pending_cast = (hn, c)

                if t + 1 < T:
                    cur[c] = x_phase(t + 1, c)

        # flush the last pending cast (h8 unused afterwards, but keep state
        # consistent)
        if pending_cast is not None:
            hn_prev, cprev = pending_cast
            h8n = h8pool[cprev].tile([128, 256], F8)
            nc.scalar.copy(h8n[:], hn_prev[:])
            h8[cprev] = h8n
            pending_cast = None

        # ---- BN + dense epilogue: y = s . h + c ----
        fin = zrpool[0].tile([128, 512], F32, tag="zr")
        first = True
        for c in (0, 1):
            for ct in (0, 1):
                nc.tensor.matmul(
                    fin[0:1, c * 128 : (c + 1) * 128],
                    sv[:, ct : ct + 1],
                    h[c][:, ct * 128 : (ct + 1) * 128],
                    start=first,
                    stop=(c == 1 and ct == 1),
                )
                first = False
        ysb = const.tile([1, 256], F32)
        nc.vector.tensor_scalar_add(ysb[:], fin[0:1, 0:256], cv[0:1, 0:1])
        nc.sync.dma_start(y_d.ap(), ysb[:])

    nc.compile()
    return nc


BN_EPS = 1e-3


def prep_core_inputs(x_core, kernel, rec_kernel, bias, gamma, beta,
                     moving_mean, moving_var, dense_w, dense_b):
    """Host-side prep of one core's input dict. x_core: [B=256, T, 32] f32."""
    import ml_dtypes

    B, T, F = x_core.shape
    H = 256
    # xT tiles: 2 timesteps per 128 rows, feats at rows 0:32 / 64:96,
    # ones-row at 32 / 96.
    xt = np.zeros((T // 2, 128, B), np.float16)
    xf = x_core.astype(np.float16).transpose(1, 2, 0)  # [T, 32, B]
    xt[:, 0:32, :] = xf[0::2]
    xt[:, 64:96, :] = xf[1::2]
    xt[:, 32, :] = 1.0
    xt[:, 96, :] = 1.0
    xT = np.ascontiguousarray(xt.reshape(T // 2 * 128, B))

    b_z = bias[0, 0:256] + bias[1, 0:256]
    b_r = bias[0, 256:512] + bias[1, 256:512]
    b_xh = bias[0, 512:768]
    b_rh = bias[1, 512:768]
    wi = np.zeros((128, 1024), np.float16)
    wi[0:32, 0:768] = kernel.astype(np.float16)
    wi[32, 0:768] = np.concatenate([b_z, b_r, b_xh]).astype(np.float16)
    wi[32, 768:1024] = b_rh.astype(np.float16)
    wi[64:97, :] = wi[0:33, :]

    rec8 = np.clip(rec_kernel, -240, 240).astype(ml_dtypes.float8_e4m3)
    wr8 = np.zeros((128, 1536), ml_dtypes.float8_e4m3)
    for m in range(6):
        for k in (0, 1):
            wr8[:, m * 256 + k * 128 : m * 256 + (k + 1) * 128] = rec8[
                k * 128 : (k + 1) * 128, m * 128 : (m + 1) * 128
            ]

    rs = 1.0 / np.sqrt(moving_var + BN_EPS)
    s = (gamma * rs * dense_w[:, 0]).astype(np.float16)
    sv = np.stack([s[:128], s[128:]], axis=1)
    cc = dense_b[0] + np.sum((beta - moving_mean * gamma * rs) * dense_w[:, 0])
    cv = np.array([[cc]], np.float32)
    return {
        "xT": xT,
        "wi": np.ascontiguousarray(wi),
        "wr8": np.ascontiguousarray(wr8),
        "sv": np.ascontiguousarray(sv),
        "cv": cv,
    }


_NC_CACHE = {}


def _get_nc():
    if "nc" not in _NC_CACHE:
        _NC_CACHE["nc"] = build_gru_nc(T=256)
    return _NC_CACHE["nc"]


def kernel(x, kernel, rec_kernel, bias, gamma, beta, moving_mean, moving_var,
           dense_w, dense_b):
    from concourse.bass_utils import run_bass_kernel_spmd

    x = np.asarray(x, dtype=np.float32)
    args = [np.asarray(a, dtype=np.float32) for a in
            (kernel, rec_kernel, bias, gamma, beta, moving_mean, moving_var,
             dense_w, dense_b)]
    nc = _get_nc()
    n_cores = 8
    nb = x.shape[0] // n_cores
    in_maps = [prep_core_inputs(x[i * nb : (i + 1) * nb], *args)
               for i in range(n_cores)]
    res = run_bass_kernel_spmd(nc, in_maps, core_ids=list(range(n_cores)))
    return np.concatenate(
        [res.results[i]["y"].reshape(nb, 1) for i in range(n_cores)], axis=0
    ).astype(np.float32)
